# revision 33
# baseline (speedup 1.0000x reference)
"""Trainium2 Bass kernel for nn_Classifier_69818988363910 (segment_reduce).

Reference computation (after dead-code elimination):
    local = relu(x @ W1^T)                        # [60000, 2048]
    feats = local.reshape(2000, 30, 2048).mean(1) # [2000, 2048]
    logits = concat(feats, feats) @ Wlin^T        # [2000, 1000]
           = feats @ (Wlin[:, :2048] + Wlin[:, 2048:])^T
y / W2 are computed but unused in the reference (original-code bug), so the
output depends only on x, W1, Wlin.

Sharding: data-parallel over the 8 NeuronCores along T (7500 rows = 250
segments per core); W1 / Wc replicated. No collectives; host gathers.

Device kernel per core (fp32 accumulation in PSUM throughout):
    MM1 on PE:   z[e, t] = sum_d W1T[d, e] * xT[d, t]
                 bf16 mode: 8 k-tiles of 128;  fp8 mode: 4 DoubleRow
                 super-k-tiles of 256 (2x PE rate)
    relu on ACT: psum -> sbuf
    pool on DVE: tensor_reduce over [128, segs, 30] view (sum; the 1/30
                 mean scale and the fp8 W1 pre-scale are folded into Wc
                 on the host)
    MM2 on PE:   logits[s, c] = sum_e featsT[e, s] * WcT[e, c]  (bf16)

Chunk 0 is issued k-outer across 8 parallel PSUM groups so the PE can
start as soon as the first weight k-tile lands, instead of waiting for
the whole W1 load.
"""

import os

import numpy as np
import ml_dtypes

BF16 = ml_dtypes.bfloat16
FP8 = ml_dtypes.float8_e4m3

MODE = os.environ.get("BASS_KERNEL_MODE", "fp8")    # "bf16" | "fp8"
W1_SCALE = 32.0                                     # fp8 mode: keep W1 out of subnormals

N_CORES = 8
T, D, E, C, J = 60000, 1024, 2048, 1000, 30
T_LOC = T // N_CORES          # 7500 rows per core
S_LOC = T_LOC // J            # 250 segments per core
CHUNK = 480                   # t-chunk (16 segments); last chunk is 300
E_TILES = E // 128            # 16
S_BLK = 125                   # MM2 output rows per block (2 blocks)
C_BLK = 500                   # MM2 output cols per chunk (2 chunks)

_cache = {}


def _build(mode):
    from concourse import bacc, mybir
    from concourse.tile import TileContext

    f32 = mybir.dt.float32
    bf16 = mybir.dt.bfloat16
    fp8 = mybir.dt.float8e4
    in_dt = fp8 if mode == "fp8" else bf16
    KT = 4 if mode == "fp8" else 8          # accumulation steps per psum group
    perf = mybir.MatmulPerfMode.DoubleRow if mode == "fp8" else None

    nc = bacc.Bacc(trn_type="TRN2", target_bir_lowering=False, debug=False,
                   num_devices=N_CORES)

    xt_d = nc.declare_dram_parameter("xt", [D, T_LOC], in_dt, isOutput=False)
    w1t_d = nc.declare_dram_parameter("w1t", [D, E], in_dt, isOutput=False)
    wct_d = nc.declare_dram_parameter("wct", [E, C], bf16, isOutput=False)
    out_d = nc.declare_dram_parameter("out", [S_LOC, C], f32, isOutput=True)

    # t-chunks: 15 x 480 + 1 x 300
    chunks = []
    t0 = 0
    while t0 < T_LOC:
        w = min(CHUNK, T_LOC - t0)
        chunks.append((t0, w))
        t0 += w

    # [D, T_LOC] viewed so a chunk DMA lands as [128p, d_tile, w]
    xt_v = xt_d[:, :].rearrange("(d p) t -> p d t", p=128)
    if mode == "fp8":
        # weight super-k-tile kt covers d = kt*256 + j*128 + p
        w1t_v = w1t_d[:, :].rearrange("(kt j p) e -> p kt j e", j=2, p=128)
    else:
        w1t_v = w1t_d[:, :].rearrange("(kt p) e -> p kt e", p=128)

    with TileContext(nc) as tc:
        with (
            tc.tile_pool(name="xin", bufs=2) as px,
            tc.tile_pool(name="wgt", bufs=1) as pw,
            tc.tile_pool(name="zrl", bufs=4) as pz,
        ):
            # --- weight tiles; issued on GpSimd's DMA queue so descriptor
            # generation runs in parallel with the x-chunk DMAs on Sync ---
            w1_sb = []
            for kt in range(KT):
                # kt=0 in quarters (gates the very first MMs); later k-tiles
                # in halves — the measured sweet spot between descriptor-gen
                # serialization and per-queue transfer bandwidth
                if kt == 0:
                    bounds = (0, 512, 1024, 1536, E)
                else:
                    bounds = (0, E // 2, E)
                pieces = [slice(a, b) for a, b in zip(bounds[:-1], bounds[1:])]
                if mode == "fp8":
                    t = pw.tile([128, 2, E], fp8, tag=f"w1_{kt}", name=f"w1_{kt}")
                    for hs in pieces:
                        nc.gpsimd.dma_start(out=t[:, :, hs],
                                            in_=w1t_v[:, kt, :, hs])
                else:
                    t = pw.tile([128, E], bf16, tag=f"w1_{kt}", name=f"w1_{kt}")
                    for hs in pieces:
                        nc.gpsimd.dma_start(out=t[:, hs], in_=w1t_v[:, kt, hs])
                w1_sb.append(t)

            feats = [pw.tile([128, S_LOC], f32, tag=f"fs_{e}", name=f"fs_{e}")
                     for e in range(E_TILES)]

            def lhsT(kt, e):
                if mode == "fp8":
                    return w1_sb[kt][:, :, e * 128:(e + 1) * 128]
                return w1_sb[kt][:, e * 128:(e + 1) * 128]

            def rhs(xt, kt, w):
                if mode == "fp8":
                    return xt[:, 2 * kt:2 * kt + 2, :w]
                return xt[:, kt, :w]

            featsb = [pw.tile([128, S_LOC], bf16, tag=f"fb_{e}", name=f"fb_{e}")
                      for e in range(E_TILES)]

            def relu_pool(ps, w, e, s0, last=False):
                segs = w // J
                zr = pz.tile([128, CHUNK], f32, tag="zr", name="zr")
                nc.scalar.activation(zr[:, :w], ps[:, :w],
                                     mybir.ActivationFunctionType.Relu)
                nc.vector.tensor_reduce(
                    out=feats[e][:, s0:s0 + segs],
                    in_=zr[:, :w].rearrange("p (s j) -> p s j", j=J),
                    axis=mybir.AxisListType.X,
                    op=mybir.AluOpType.add,
                )
                if last:
                    # feats[e] is complete once the last chunk's pool ran;
                    # convert for MM2 right away so MM2 never waits.
                    nc.vector.tensor_copy(featsb[e], feats[e])

            wc_sb = None

            with tc.tile_pool(name="ps1", bufs=8, space="PSUM") as pp1:
                n_dt = 2 * KT if mode == "fp8" else KT
                for ci, (t0, w) in enumerate(chunks):
                    xt = px.tile([128, n_dt, CHUNK], in_dt, tag="xt", name="xt")
                    for d in range(n_dt):
                        # chunk 0: spread descriptor generation over two
                        # engines so the first k-slices land sooner
                        eng = nc.scalar if (ci == 0 and d >= n_dt // 2) else nc.sync
                        eng.dma_start(out=xt[:, d, :w],
                                      in_=xt_v[:, d, t0:t0 + w])
                    s0 = t0 // J
                    if ci == 0:
                        # k-outer across parallel psum groups: first MMs
                        # only need w1_sb[0] + the first x k-slices.
                        e0 = 0
                        for wave in (8, 8):
                            pss = [pp1.tile([128, CHUNK], f32, tag="ps",
                                            name=f"ps0_{e0}_{i}")
                                   for i in range(wave)]
                            for kt in range(KT):
                                for i in range(wave):
                                    nc.tensor.matmul(
                                        pss[i][:, :w],
                                        lhsT(kt, e0 + i),
                                        rhs(xt, kt, w),
                                        start=(kt == 0),
                                        stop=(kt == KT - 1),
                                        perf_mode=perf,
                                    )
                            for i in range(wave):
                                relu_pool(pss[i], w, e0 + i, s0)
                            e0 += wave
                        continue
                    for e in range(E_TILES):
                        ps = pp1.tile([128, CHUNK], f32, tag="ps", name="ps")
                        for kt in range(KT):
                            nc.tensor.matmul(
                                ps[:, :w],
                                lhsT(kt, e),
                                rhs(xt, kt, w),
                                start=(kt == 0),
                                stop=(kt == KT - 1),
                                perf_mode=perf,
                            )
                        relu_pool(ps, w, e, s0, last=(ci == len(chunks) - 1))
                    if ci == 1:
                        # MM2 weights: issued late so they don't compete
                        # with W1/x for startup bandwidth.
                        wc_sb = []
                        for e in range(E_TILES):
                            t = pw.tile([128, C], bf16, tag=f"wc_{e}",
                                        name=f"wc_{e}")
                            nc.gpsimd.dma_start(
                                out=t, in_=wct_d[e * 128:(e + 1) * 128, :])
                            wc_sb.append(t)

                for sb in range(S_LOC // S_BLK):
                    ob = pw.tile([S_BLK, C], f32, tag=f"ob_{sb}", name=f"ob_{sb}")
                    for c0, cw in ((0, 500), (500, 500)):
                        ps = pp1.tile([S_BLK, C_BLK], f32, tag="ps", name="ps2")
                        for e in range(E_TILES):
                            nc.tensor.matmul(
                                ps[:, :cw],
                                featsb[e][:, sb * S_BLK:(sb + 1) * S_BLK],
                                wc_sb[e][:, c0:c0 + cw],
                                start=(e == 0),
                                stop=(e == E_TILES - 1),
                            )
                        cs = slice(c0, c0 + cw)
                        nc.scalar.copy(ob[:, cs], ps[:, :cw])
                        nc.scalar.dma_start(
                            out=out_d[sb * S_BLK:(sb + 1) * S_BLK, cs],
                            in_=ob[:, cs])

    nc.compile()
    return nc


def _prep_inputs(x, W1, Wlin, mode=MODE):
    wc = (Wlin[:, :E] + Wlin[:, E:]) / np.float32(J)     # [C, E] f32
    if mode == "fp8":
        in_np = FP8
        W1 = W1 * np.float32(W1_SCALE)
        wc = wc / np.float32(W1_SCALE)
    else:
        in_np = BF16
    wct = np.ascontiguousarray(wc.T).astype(BF16)        # [E, C] bf16
    w1t = np.ascontiguousarray(W1.T).astype(in_np)       # [D, E]
    in_maps = []
    for c in range(N_CORES):
        xs = x[c * T_LOC:(c + 1) * T_LOC]                # [7500, 1024]
        xt = np.ascontiguousarray(xs.T).astype(in_np)    # [1024, 7500]
        in_maps.append({"xt": xt, "w1t": w1t, "wct": wct})
    return in_maps


def _run(in_maps, mode=MODE, trace=False, **kw):
    from concourse.bass_utils import run_bass_kernel_spmd

    if mode not in _cache:
        _cache[mode] = _build(mode)
    res = run_bass_kernel_spmd(_cache[mode], in_maps,
                               core_ids=list(range(N_CORES)), trace=trace, **kw)
    logits = np.concatenate([r["out"] for r in res.results], axis=0)
    return logits, res


def kernel(x, y, W1, W2, Wlin):
    x = np.asarray(x, dtype=np.float32)
    W1 = np.asarray(W1, dtype=np.float32)
    Wlin = np.asarray(Wlin, dtype=np.float32)
    modes = (MODE, "bf16") if MODE != "bf16" else ("bf16",)
    for i, mode in enumerate(modes):
        try:
            logits, _ = _run(_prep_inputs(x, W1, Wlin, mode=mode), mode=mode)
            return logits
        except Exception:
            if i == len(modes) - 1:
                raise
    raise RuntimeError("unreachable")
